# revision 20
# baseline (speedup 1.0000x reference)
"""Causal multi-head self-attention on 8 trn2 NeuronCores.

Sharding: core c = (batch b = c//2, head-group g = c%2). Each core handles one
batch element and 6 of the 12 heads: QKV projection for its 384 output dims,
causal attention for its 6 heads, and a partial output projection against the
matching 384 columns of o_proj. Host sums the two partials per batch.

Device-side layout (per core):
  xT  [768, 2048]   x transposed (host-side), d on partitions
  QT/KT pair tiles [128, 2048]: partitions = (head 2p | head 2p+1) x dk=64,
    free dim = sequence. Produced by out = wT.T @ xT matmuls.
  S^T tiles [k, q]: scores transposed, computed with head-pair row tiling
    (K=dk=64 per head, two heads in array rows 0-63 / 64-127).
  exp on ScalarE (PSUM -> SBUF, bf16), causal mask applied as a 0/1 multiply
    on the diagonal blocks only.
  V_aug [k, 65]: V for one head + ones column; A@V matmul then yields both
    O^T (rows 0..63) and the softmax denominator (row 64) in one chain.
  Normalization: denom reciprocal (DVE), broadcast across partitions via a
    K=1 ones matmul on PE, multiply on DVE -> OT tiles [c, s].
  Output projection: out = OT.T @ owT accumulated over the 3 c-blocks.
"""

import numpy as np
import ml_dtypes

B, S, D = 4, 2048, 768
H, DK = 12, 64
NCORES = 8
GH = 6        # heads per core
GO = GH * DK  # 384, per-core slice of the qkv output dim
NP = 3        # head pairs per core
NSB = S // 128   # 16 sequence blocks of 128
NJ = S // 512    # 4 q-chunks of 512

BF16 = ml_dtypes.bfloat16

_CACHE = {}


def _build_bass():
    import concourse.bass as bass  # noqa: F401
    import concourse.tile as tile
    from concourse import bacc, mybir
    from contextlib import ExitStack

    f32 = mybir.dt.float32
    bf16 = mybir.dt.bfloat16
    AF = mybir.ActivationFunctionType

    nc = bacc.Bacc("TRN2", target_bir_lowering=False, debug=False,
                   num_devices=NCORES)

    xT_d = nc.dram_tensor("xT", [D, S], bf16, kind="ExternalInput").ap()
    wqT_d = nc.dram_tensor("wqT", [D, GO], bf16, kind="ExternalInput").ap()
    wkT_d = nc.dram_tensor("wkT", [D, GO], bf16, kind="ExternalInput").ap()
    wvT_d = nc.dram_tensor("wvT", [D, GO], bf16, kind="ExternalInput").ap()
    owT_d = nc.dram_tensor("owT", [GO, D], bf16, kind="ExternalInput").ap()
    mk_d = nc.dram_tensor("mk", [128, 128], bf16, kind="ExternalInput").ap()
    sel_d = nc.dram_tensor("sel", [GH, GH * 64], f32, kind="ExternalInput").ap()
    part_d = nc.dram_tensor("part", [S, D], f32, kind="ExternalOutput").ap()

    ND = D // 128  # 6 d-blocks

    with tile.TileContext(nc) as tc, ExitStack() as ctx:
        pers = ctx.enter_context(tc.tile_pool(name="pers", bufs=1))

        # ---- persistent SBUF tiles -------------------------------------
        xT = [pers.tile([128, S], bf16, tag=f"xT{d}", name=f"xT{d}") for d in range(ND)]
        wq = [pers.tile([128, GO], bf16, tag=f"wq{d}", name=f"wq{d}") for d in range(ND)]
        wk = [pers.tile([128, GO], bf16, tag=f"wk{d}", name=f"wk{d}") for d in range(ND)]
        wv = [pers.tile([128, GO], bf16, tag=f"wv{d}", name=f"wv{d}") for d in range(ND)]
        ow = [pers.tile([128, D], bf16, tag=f"ow{c}", name=f"ow{c}") for c in range(NP)]
        mkt = pers.tile([128, 128], bf16, tag="mkt", name="mkt")
        QT = [pers.tile([128, S], bf16, tag=f"QT{p}", name=f"QT{p}") for p in range(NP)]
        KT = [pers.tile([128, S], bf16, tag=f"KT{p}", name=f"KT{p}") for p in range(NP)]
        OT = [pers.tile([128, S], bf16, tag=f"OT{p}", name=f"OT{p}") for p in range(NP)]
        vaug = [pers.tile([128, GH * 65], bf16, tag=f"va{kb}", name=f"va{kb}")
                for kb in range(NSB)]
        # selector matrix (host-built): sel_t[k, h*64+m] = (k == h); the
        # [6, 64] slice for head h is the lhsT of a K=6 matmul that
        # broadcasts reciprocal row h across 64 output partitions
        sel_t = pers.tile([GH, GH * 64], f32, tag="sel_t", name="sel_t")
        nc.sync.dma_start(sel_t[:], sel_d[:])

        # DMA priority: what the first score matmuls need lands first —
        # wq/wk, then the j=3 x columns, then the rest in reverse-j order.
        for d in range(ND):
            nc.sync.dma_start(wq[d][:], wqT_d[d * 128:(d + 1) * 128, :])
            nc.sync.dma_start(wk[d][:], wkT_d[d * 128:(d + 1) * 128, :])
        for jj in reversed(range(NJ)):
            for d in range(ND):
                nc.sync.dma_start(xT[d][:, jj * 512:(jj + 1) * 512],
                                  xT_d[d * 128:(d + 1) * 128,
                                       jj * 512:(jj + 1) * 512])
        nc.sync.dma_start(mkt[:], mk_d[:])
        for d in range(ND):
            nc.sync.dma_start(wv[d][:], wvT_d[d * 128:(d + 1) * 128, :])
        for c in range(NP):
            nc.sync.dma_start(ow[c][:], owT_d[c * 128:(c + 1) * 128, :])

        # one unified PSUM pool: sp = scores strip (4 banks), u512 = shared
        # 1-bank slots for proj / AV / broadcast / output projection
        psp = ctx.enter_context(tc.tile_pool(name="psp", space="PSUM", bufs=2))
        pav = ctx.enter_context(tc.tile_pool(name="pav", space="PSUM", bufs=2))
        pu = ctx.enter_context(tc.tile_pool(name="pu", space="PSUM", bufs=2))
        expp = ctx.enter_context(tc.tile_pool(name="expp", bufs=4))
        sml = ctx.enter_context(tc.tile_pool(name="sml", bufs=2))
        stg = ctx.enter_context(tc.tile_pool(name="stg", bufs=3))

        def proj_qk(wsrc, dst, ob, j):
            ps = pu.tile([128, 512], f32, tag="u512", name="pjq")
            for d in range(ND):
                nc.tensor.matmul(ps[:], wsrc[d][:, ob * 128:(ob + 1) * 128],
                                 xT[d][:, j * 512:(j + 1) * 512],
                                 start=(d == 0), stop=(d == ND - 1))
            nc.vector.tensor_copy(dst[ob][:, j * 512:(j + 1) * 512], ps[:])

        def proj_v(sb):
            ps = pu.tile([128, GO], f32, tag="u512", name="pjv")
            for d in range(ND):
                nc.tensor.matmul(ps[:], xT[d][:, sb * 128:(sb + 1) * 128],
                                 wv[d][:], start=(d == 0), stop=(d == ND - 1))
            vt = vaug[sb]
            dst = vt[:].rearrange("p (h c) -> p h c", h=GH)[:, :, 0:64]
            nc.vector.tensor_copy(
                dst, ps[:].rearrange("p (h c) -> p h c", h=GH))
            nc.vector.memset(
                vt[:].rearrange("p (h c) -> p h c", h=GH)[:, :, 64:65], 1.0)

        # ---- attention, software-pipelined --------------------------------
        # One serial exp chain on ScalarE is the backbone. Per kb-group:
        # two score matmuls (head pair, row-tiled) -> exp -> mask; the AV
        # matmuls for group kb-1 run while exp(kb) drains, so PE's in-order
        # stream never waits on the chain. Filler thunks (V/QK projections,
        # normalization, output projection) are paced between groups.
        def attention_pair(j, p, dn, filler):
            nkb = 4 * j + 4
            nfill = len(filler)
            stride = max(1, -(-nkb // (nfill + 1))) if nfill else nkb + 1
            av = [pav.tile([65, 512], f32, tag="av", name="av")
                  for _ in range(2)]
            pend = None
            fi = 0
            for kb in range(nkb):
                sp = psp.tile([128, 1024], f32, tag="sp", name="sp")
                ex = expp.tile([128, 1024], bf16, tag="ex", name="ex")
                for hh in range(2):
                    nc.tensor.matmul(
                        sp[:, hh * 512:(hh + 1) * 512],
                        KT[p][hh * 64:(hh + 1) * 64,
                              kb * 128:(kb + 1) * 128],
                        QT[p][hh * 64:(hh + 1) * 64,
                              j * 512:(j + 1) * 512],
                        start=True, stop=True,
                        tile_position=(hh * 64, 0))
                nc.scalar.activation(ex[:], sp[:], AF.Exp)
                v = kb - 4 * j
                if v >= 0:  # diagonal block: causal mask
                    for hh in range(2):
                        if v > 0:
                            nc.vector.memset(
                                ex[:, hh * 512:hh * 512 + 128 * v], 0.0)
                        tri = ex[:, hh * 512 + 128 * v:
                                 hh * 512 + 128 * (v + 1)]
                        nc.vector.tensor_mul(tri, tri, mkt[:])
                if pend is not None:
                    pkb, pex = pend
                    for hh in range(2):
                        nc.tensor.matmul(
                            av[hh][:],
                            vaug[pkb][:, (2 * p + hh) * 65:
                                      (2 * p + hh + 1) * 65],
                            pex[:, hh * 512:(hh + 1) * 512],
                            start=(pkb == 0), stop=False)
                if kb % stride == stride - 1 and fi < nfill:
                    filler[fi]()
                    fi += 1
                pend = (kb, ex)
            pkb, pex = pend
            for hh in range(2):
                nc.tensor.matmul(
                    av[hh][:],
                    vaug[pkb][:, (2 * p + hh) * 65:(2 * p + hh + 1) * 65],
                    pex[:, hh * 512:(hh + 1) * 512],
                    start=(pkb == 0), stop=True)
            while fi < nfill:
                filler[fi]()
                fi += 1
            for hh in range(2):
                h = 2 * p + hh
                nc.vector.tensor_copy(
                    OT[p][hh * 64:(hh + 1) * 64, j * 512:(j + 1) * 512],
                    av[hh][0:64, :])
                ds_ = sml.tile([1, 512], f32, tag="dstg", name="dstg",
                               bufs=6)
                nc.vector.tensor_copy(ds_[:], av[hh][64:65, :])
                nc.sync.dma_start(dn[h:h + 1, :], ds_[:])

        def norm_thunks(j, dn):
            rcp6 = sml.tile([6, 512], f32, tag="rcp6", name="rcp6")
            nc.vector.reciprocal(rcp6[:], dn[:])
            out = []

            def mk_bc(p):
                def f():
                    bc = pu.tile([128, 512], f32, tag="u512", name="bc")
                    for hh in range(2):
                        h = 2 * p + hh
                        nc.tensor.matmul(bc[hh * 64:(hh + 1) * 64, :],
                                         sel_t[:, h * 64:(h + 1) * 64],
                                         rcp6[:], start=True, stop=True,
                                         tile_position=(0, hh * 64))
                    nc.vector.tensor_mul(OT[p][:, j * 512:(j + 1) * 512],
                                         OT[p][:, j * 512:(j + 1) * 512],
                                         bc[:])
                return f

            def mk_op(sb):
                def f():
                    op1 = pu.tile([128, 512], f32, tag="u512", name="op1")
                    op2 = pu.tile([128, 256], f32, tag="u512", name="op2")
                    for cb in range(NP):
                        lhs = OT[cb][:, sb * 128:(sb + 1) * 128]
                        nc.tensor.matmul(op1[:], lhs, ow[cb][:, 0:512],
                                         start=(cb == 0), stop=(cb == NP - 1))
                        nc.tensor.matmul(op2[:], lhs, ow[cb][:, 512:768],
                                         start=(cb == 0), stop=(cb == NP - 1))
                    st = stg.tile([128, D], f32, tag="st", name="st")
                    nc.vector.tensor_copy(st[:, 0:512], op1[:])
                    nc.vector.tensor_copy(st[:, 512:768], op2[:])
                    nc.sync.dma_start(part_d[sb * 128:(sb + 1) * 128, :],
                                      st[:])
                return f

            for p in range(NP):
                out.append(mk_bc(p))
            for sb in range(4 * j, 4 * j + 4):
                out.append(mk_op(sb))
            return out

        for j in reversed(range(NJ)):
            proj_qk(wq, QT, 0, j)
            proj_qk(wk, KT, 0, j)

        vthunks = [(lambda sb=sb: proj_v(sb)) for sb in range(NSB)]
        qk1 = [(lambda jj=jj: (proj_qk(wq, QT, 1, jj),
                               proj_qk(wk, KT, 1, jj)))
               for jj in reversed(range(NJ))]
        qk2 = [(lambda jj=jj: (proj_qk(wq, QT, 2, jj),
                               proj_qk(wk, KT, 2, jj)))
               for jj in reversed(range(NJ))]

        prev = None
        for idx, j in enumerate(reversed(range(NJ))):
            dn = sml.tile([6, 512], f32, tag="dn", name="dn")
            if idx == 0:
                # producers must be emitted before their consumers: pair-1's
                # Q/K chains drain inside pair 0, pair-2's inside pair 1.
                fill0, fill1, fill2 = vthunks + qk1, qk2, []
            else:
                nt = norm_thunks(prev[0], prev[1])
                fill0, fill1, fill2 = nt[:3], nt[3:5], nt[5:]
            attention_pair(j, 0, dn, fill0)
            attention_pair(j, 1, dn, fill1)
            attention_pair(j, 2, dn, fill2)
            prev = (j, dn)
        for t in norm_thunks(prev[0], prev[1]):
            t()

    nc.compile()
    return nc


def _prep_in_maps(in_features, qkv_proj_weight, o_proj_weight):
    """Per-core input dict (host-side shard + transpose + cast)."""
    # causal 0/1 triangle for diagonal 128x128 blocks
    r = np.arange(128)[:, None]
    c = np.arange(128)[None, :]
    mk = (r <= c).astype(np.float32).astype(BF16)
    sel = np.zeros((GH, GH * 64), np.float32)
    for h in range(GH):
        sel[h, h * 64:(h + 1) * 64] = 1.0

    scale = 1.0 / np.sqrt(np.float32(DK))
    in_maps = []
    for core in range(NCORES):
        b, g = core // 2, core % 2
        sl = slice(g * GO, (g + 1) * GO)
        xT = np.ascontiguousarray(in_features[b].T).astype(BF16)
        wqT = np.ascontiguousarray((qkv_proj_weight[0][sl, :] * scale).T
                                   ).astype(BF16)
        wkT = np.ascontiguousarray(qkv_proj_weight[1][sl, :].T).astype(BF16)
        wvT = np.ascontiguousarray(qkv_proj_weight[2][sl, :].T).astype(BF16)
        owT = np.ascontiguousarray(o_proj_weight[:, sl].T).astype(BF16)
        in_maps.append({"xT": xT, "wqT": wqT, "wkT": wkT, "wvT": wvT,
                        "owT": owT, "mk": mk, "sel": sel})
    return in_maps


def _get_runner():
    """Persistent sharded-jit runner over the 8 NeuronCores.

    Mirrors bass_utils.run_bass_kernel_spmd's axon path
    (bass2jax.run_bass_via_pjrt), but keeps the jitted executable cached
    across calls and skips install_neuronx_cc_hook: under axon the
    bass_exec custom-call is compiled terminal-side, and the client-side
    hook rejects the SPMD-partitioned HLO.
    """
    if "runner" in _CACHE:
        return _CACHE["runner"]

    import jax
    from concourse import mybir
    from concourse.bass2jax import _bass_exec_p, partition_id_tensor
    from jax.sharding import Mesh, PartitionSpec
    from jax.experimental.shard_map import shard_map

    nc = _build_bass()

    partition_name = (nc.partition_id_tensor.name
                      if nc.partition_id_tensor else None)
    in_names, out_names, out_avals, zero_outs = [], [], [], []
    for alloc in nc.m.functions[0].allocations:
        if not isinstance(alloc, mybir.MemoryLocationSet):
            continue
        name = alloc.memorylocations[0].name
        if alloc.kind == "ExternalInput":
            if name != partition_name:
                in_names.append(name)
        elif alloc.kind == "ExternalOutput":
            out_names.append(name)
            shape = tuple(alloc.tensor_shape)
            dtype = mybir.dt.np(alloc.dtype)
            out_avals.append(jax.core.ShapedArray(shape, dtype))
            zero_outs.append(np.zeros(shape, dtype))
    n_params = len(in_names)
    n_outs = len(out_avals)
    all_in = list(in_names) + out_names + (
        [partition_name] if partition_name else [])

    def _body(*args):
        operands = list(args)
        if partition_name is not None:
            operands.append(partition_id_tensor())
        return tuple(_bass_exec_p.bind(
            *operands,
            out_avals=tuple(out_avals),
            in_names=tuple(all_in),
            out_names=tuple(out_names),
            lowering_input_output_aliases=(),
            sim_require_finite=True, sim_require_nnan=True, nc=nc))

    devices = jax.devices()[:NCORES]
    mesh = Mesh(np.asarray(devices), ("core",))
    fn = jax.jit(
        shard_map(_body, mesh=mesh,
                  in_specs=(PartitionSpec("core"),) * (n_params + n_outs),
                  out_specs=(PartitionSpec("core"),) * n_outs,
                  check_rep=False),
        donate_argnums=tuple(range(n_params, n_params + n_outs)),
        keep_unused=True)

    def run(in_maps):
        per_core = [[np.asarray(m[n]) for n in in_names] for m in in_maps]
        concat_in = [np.concatenate([per_core[c][i] for c in range(NCORES)],
                                    axis=0) for i in range(n_params)]
        concat_zeros = [np.zeros((NCORES * z.shape[0], *z.shape[1:]), z.dtype)
                        for z in zero_outs]
        out_arrs = fn(*concat_in, *concat_zeros)
        return np.asarray(out_arrs[out_names.index("part")]).reshape(
            NCORES, S, D)

    _CACHE["nc"] = nc
    _CACHE["runner"] = run
    return run


def kernel(in_features, qkv_proj_weight, o_proj_weight):
    run = _get_runner()
    in_maps = _prep_in_maps(np.asarray(in_features, np.float32),
                            np.asarray(qkv_proj_weight, np.float32),
                            np.asarray(o_proj_weight, np.float32))
    parts = run(in_maps)
    out = np.empty((B, S, D), np.float32)
    for b in range(B):
        out[b] = parts[2 * b] + parts[2 * b + 1]
    return out


# revision 21
# speedup vs baseline: 1.0087x; 1.0087x over previous
"""Causal multi-head self-attention on 8 trn2 NeuronCores.

Sharding: core c = (batch b = c//2, head-group g = c%2). Each core handles one
batch element and 6 of the 12 heads: QKV projection for its 384 output dims,
causal attention for its 6 heads, and a partial output projection against the
matching 384 columns of o_proj. Host sums the two partials per batch.

Device-side layout (per core):
  xT  [768, 2048]   x transposed (host-side), d on partitions
  QT/KT pair tiles [128, 2048]: partitions = (head 2p | head 2p+1) x dk=64,
    free dim = sequence. Produced by out = wT.T @ xT matmuls.
  S^T tiles [k, q]: scores transposed, computed with head-pair row tiling
    (K=dk=64 per head, two heads in array rows 0-63 / 64-127).
  exp on ScalarE (PSUM -> SBUF, bf16), causal mask applied as a 0/1 multiply
    on the diagonal blocks only.
  V_aug [k, 65]: V for one head + ones column; A@V matmul then yields both
    O^T (rows 0..63) and the softmax denominator (row 64) in one chain.
  Normalization: denom reciprocal (DVE), broadcast across partitions via a
    K=1 ones matmul on PE, multiply on DVE -> OT tiles [c, s].
  Output projection: out = OT.T @ owT accumulated over the 3 c-blocks.
"""

import numpy as np
import ml_dtypes

B, S, D = 4, 2048, 768
H, DK = 12, 64
NCORES = 8
GH = 6        # heads per core
GO = GH * DK  # 384, per-core slice of the qkv output dim
NP = 3        # head pairs per core
NSB = S // 128   # 16 sequence blocks of 128
NJ = S // 512    # 4 q-chunks of 512

BF16 = ml_dtypes.bfloat16

_CACHE = {}


def _build_bass():
    import concourse.bass as bass  # noqa: F401
    import concourse.tile as tile
    from concourse import bacc, mybir
    from contextlib import ExitStack

    f32 = mybir.dt.float32
    bf16 = mybir.dt.bfloat16
    AF = mybir.ActivationFunctionType

    nc = bacc.Bacc("TRN2", target_bir_lowering=False, debug=False,
                   num_devices=NCORES)

    xT_d = nc.dram_tensor("xT", [D, S], bf16, kind="ExternalInput").ap()
    wqT_d = nc.dram_tensor("wqT", [D, GO], bf16, kind="ExternalInput").ap()
    wkT_d = nc.dram_tensor("wkT", [D, GO], bf16, kind="ExternalInput").ap()
    wvT_d = nc.dram_tensor("wvT", [D, GO], bf16, kind="ExternalInput").ap()
    owT_d = nc.dram_tensor("owT", [GO, D], bf16, kind="ExternalInput").ap()
    mk_d = nc.dram_tensor("mk", [128, 128], bf16, kind="ExternalInput").ap()
    sel_d = nc.dram_tensor("sel", [GH, GH * 64], f32, kind="ExternalInput").ap()
    part_d = nc.dram_tensor("part", [S, D], f32, kind="ExternalOutput").ap()

    ND = D // 128  # 6 d-blocks

    with tile.TileContext(nc) as tc, ExitStack() as ctx:
        pers = ctx.enter_context(tc.tile_pool(name="pers", bufs=1))

        # ---- persistent SBUF tiles -------------------------------------
        xT = [pers.tile([128, S], bf16, tag=f"xT{d}", name=f"xT{d}") for d in range(ND)]
        wq = [pers.tile([128, GO], bf16, tag=f"wq{d}", name=f"wq{d}") for d in range(ND)]
        wk = [pers.tile([128, GO], bf16, tag=f"wk{d}", name=f"wk{d}") for d in range(ND)]
        wv = [pers.tile([128, GO], bf16, tag=f"wv{d}", name=f"wv{d}") for d in range(ND)]
        ow = [pers.tile([128, D], bf16, tag=f"ow{c}", name=f"ow{c}") for c in range(NP)]
        mkt = pers.tile([128, 128], bf16, tag="mkt", name="mkt")
        QT = [pers.tile([128, S], bf16, tag=f"QT{p}", name=f"QT{p}") for p in range(NP)]
        KT = [pers.tile([128, S], bf16, tag=f"KT{p}", name=f"KT{p}") for p in range(NP)]
        OT = [pers.tile([128, S], bf16, tag=f"OT{p}", name=f"OT{p}") for p in range(NP)]
        vaug = [pers.tile([128, GH * 65], bf16, tag=f"va{kb}", name=f"va{kb}")
                for kb in range(NSB)]
        # selector matrix (host-built): sel_t[k, h*64+m] = (k == h); the
        # [6, 64] slice for head h is the lhsT of a K=6 matmul that
        # broadcasts reciprocal row h across 64 output partitions
        sel_t = pers.tile([GH, GH * 64], f32, tag="sel_t", name="sel_t")
        nc.sync.dma_start(sel_t[:], sel_d[:])

        # DMA priority: what the first score matmuls need lands first —
        # wq/wk, then the j=3 x columns, then the rest in reverse-j order.
        for d in range(ND):
            nc.sync.dma_start(wq[d][:], wqT_d[d * 128:(d + 1) * 128, :])
            nc.sync.dma_start(wk[d][:], wkT_d[d * 128:(d + 1) * 128, :])
            nc.gpsimd.dma_start(xT[d][:, 3 * 512:4 * 512],
                                xT_d[d * 128:(d + 1) * 128, 3 * 512:4 * 512])
        for d in range(ND):
            nc.gpsimd.dma_start(wv[d][:], wvT_d[d * 128:(d + 1) * 128, :])
        nc.sync.dma_start(mkt[:], mk_d[:])
        for jj in (2, 1, 0):
            for d in range(ND):
                nc.sync.dma_start(xT[d][:, jj * 512:(jj + 1) * 512],
                                  xT_d[d * 128:(d + 1) * 128,
                                       jj * 512:(jj + 1) * 512])
        for c in range(NP):
            nc.gpsimd.dma_start(ow[c][:], owT_d[c * 128:(c + 1) * 128, :])

        # one unified PSUM pool: sp = scores strip (4 banks), u512 = shared
        # 1-bank slots for proj / AV / broadcast / output projection
        psp = ctx.enter_context(tc.tile_pool(name="psp", space="PSUM", bufs=2))
        pav = ctx.enter_context(tc.tile_pool(name="pav", space="PSUM", bufs=2))
        pu = ctx.enter_context(tc.tile_pool(name="pu", space="PSUM", bufs=2))
        expp = ctx.enter_context(tc.tile_pool(name="expp", bufs=4))
        sml = ctx.enter_context(tc.tile_pool(name="sml", bufs=2))
        stg = ctx.enter_context(tc.tile_pool(name="stg", bufs=3))

        def proj_qk(wsrc, dst, ob, j):
            ps = pu.tile([128, 512], f32, tag="u512", name="pjq")
            for d in range(ND):
                nc.tensor.matmul(ps[:], wsrc[d][:, ob * 128:(ob + 1) * 128],
                                 xT[d][:, j * 512:(j + 1) * 512],
                                 start=(d == 0), stop=(d == ND - 1))
            nc.vector.tensor_copy(dst[ob][:, j * 512:(j + 1) * 512], ps[:])

        def proj_v(sb):
            ps = pu.tile([128, GO], f32, tag="u512", name="pjv")
            for d in range(ND):
                nc.tensor.matmul(ps[:], xT[d][:, sb * 128:(sb + 1) * 128],
                                 wv[d][:], start=(d == 0), stop=(d == ND - 1))
            vt = vaug[sb]
            dst = vt[:].rearrange("p (h c) -> p h c", h=GH)[:, :, 0:64]
            nc.vector.tensor_copy(
                dst, ps[:].rearrange("p (h c) -> p h c", h=GH))
            nc.vector.memset(
                vt[:].rearrange("p (h c) -> p h c", h=GH)[:, :, 64:65], 1.0)

        # ---- attention, software-pipelined --------------------------------
        # One serial exp chain on ScalarE is the backbone. Per kb-group:
        # two score matmuls (head pair, row-tiled) -> exp -> mask; the AV
        # matmuls for group kb-1 run while exp(kb) drains, so PE's in-order
        # stream never waits on the chain. Filler thunks (V/QK projections,
        # normalization, output projection) are paced between groups.
        def attention_pair(j, p, dn, filler):
            nkb = 4 * j + 4
            nfill = len(filler)
            stride = max(1, -(-nkb // (nfill + 1))) if nfill else nkb + 1
            av = [pav.tile([65, 512], f32, tag="av", name="av")
                  for _ in range(2)]
            pend = None
            fi = 0
            for kb in range(nkb):
                sp = psp.tile([128, 1024], f32, tag="sp", name="sp")
                ex = expp.tile([128, 1024], bf16, tag="ex", name="ex")
                for hh in range(2):
                    nc.tensor.matmul(
                        sp[:, hh * 512:(hh + 1) * 512],
                        KT[p][hh * 64:(hh + 1) * 64,
                              kb * 128:(kb + 1) * 128],
                        QT[p][hh * 64:(hh + 1) * 64,
                              j * 512:(j + 1) * 512],
                        start=True, stop=True,
                        tile_position=(hh * 64, 0))
                nc.scalar.activation(ex[:], sp[:], AF.Exp)
                v = kb - 4 * j
                if v >= 0:  # diagonal block: causal mask
                    for hh in range(2):
                        if v > 0:
                            nc.vector.memset(
                                ex[:, hh * 512:hh * 512 + 128 * v], 0.0)
                        tri = ex[:, hh * 512 + 128 * v:
                                 hh * 512 + 128 * (v + 1)]
                        nc.vector.tensor_mul(tri, tri, mkt[:])
                if pend is not None:
                    pkb, pex = pend
                    for hh in range(2):
                        nc.tensor.matmul(
                            av[hh][:],
                            vaug[pkb][:, (2 * p + hh) * 65:
                                      (2 * p + hh + 1) * 65],
                            pex[:, hh * 512:(hh + 1) * 512],
                            start=(pkb == 0), stop=False)
                if kb % stride == stride - 1 and fi < nfill:
                    filler[fi]()
                    fi += 1
                pend = (kb, ex)
            pkb, pex = pend
            for hh in range(2):
                nc.tensor.matmul(
                    av[hh][:],
                    vaug[pkb][:, (2 * p + hh) * 65:(2 * p + hh + 1) * 65],
                    pex[:, hh * 512:(hh + 1) * 512],
                    start=(pkb == 0), stop=True)
            while fi < nfill:
                filler[fi]()
                fi += 1
            for hh in range(2):
                h = 2 * p + hh
                nc.vector.tensor_copy(
                    OT[p][hh * 64:(hh + 1) * 64, j * 512:(j + 1) * 512],
                    av[hh][0:64, :])
                ds_ = sml.tile([1, 512], f32, tag="dstg", name="dstg",
                               bufs=6)
                nc.vector.tensor_copy(ds_[:], av[hh][64:65, :])
                nc.sync.dma_start(dn[h:h + 1, :], ds_[:])

        def norm_thunks(j, dn):
            rcp6 = sml.tile([6, 512], f32, tag="rcp6", name="rcp6")
            nc.vector.reciprocal(rcp6[:], dn[:])
            out = []

            def mk_bc(p):
                def f():
                    bc = pu.tile([128, 512], f32, tag="u512", name="bc")
                    for hh in range(2):
                        h = 2 * p + hh
                        nc.tensor.matmul(bc[hh * 64:(hh + 1) * 64, :],
                                         sel_t[:, h * 64:(h + 1) * 64],
                                         rcp6[:], start=True, stop=True,
                                         tile_position=(0, hh * 64))
                    nc.vector.tensor_mul(OT[p][:, j * 512:(j + 1) * 512],
                                         OT[p][:, j * 512:(j + 1) * 512],
                                         bc[:])
                return f

            def mk_op(sb):
                def f():
                    op1 = pu.tile([128, 512], f32, tag="u512", name="op1")
                    op2 = pu.tile([128, 256], f32, tag="u512", name="op2")
                    for cb in range(NP):
                        lhs = OT[cb][:, sb * 128:(sb + 1) * 128]
                        nc.tensor.matmul(op1[:], lhs, ow[cb][:, 0:512],
                                         start=(cb == 0), stop=(cb == NP - 1))
                        nc.tensor.matmul(op2[:], lhs, ow[cb][:, 512:768],
                                         start=(cb == 0), stop=(cb == NP - 1))
                    st = stg.tile([128, D], f32, tag="st", name="st")
                    nc.vector.tensor_copy(st[:, 0:512], op1[:])
                    nc.vector.tensor_copy(st[:, 512:768], op2[:])
                    nc.sync.dma_start(part_d[sb * 128:(sb + 1) * 128, :],
                                      st[:])
                return f

            for p in range(NP):
                out.append(mk_bc(p))
            for sb in range(4 * j, 4 * j + 4):
                out.append(mk_op(sb))
            return out

        for j in reversed(range(NJ)):
            proj_qk(wq, QT, 0, j)
            proj_qk(wk, KT, 0, j)

        vthunks = [(lambda sb=sb: proj_v(sb)) for sb in range(NSB)]
        qk1 = [(lambda jj=jj: (proj_qk(wq, QT, 1, jj),
                               proj_qk(wk, KT, 1, jj)))
               for jj in reversed(range(NJ))]
        qk2 = [(lambda jj=jj: (proj_qk(wq, QT, 2, jj),
                               proj_qk(wk, KT, 2, jj)))
               for jj in reversed(range(NJ))]

        prev = None
        for idx, j in enumerate(reversed(range(NJ))):
            dn = sml.tile([6, 512], f32, tag="dn", name="dn")
            if idx == 0:
                # producers must be emitted before their consumers: pair-1's
                # Q/K chains drain inside pair 0, pair-2's inside pair 1.
                fill0, fill1, fill2 = vthunks + qk1, qk2, []
            else:
                nt = norm_thunks(prev[0], prev[1])
                fill0, fill1, fill2 = nt[:3], nt[3:5], nt[5:]
            attention_pair(j, 0, dn, fill0)
            attention_pair(j, 1, dn, fill1)
            attention_pair(j, 2, dn, fill2)
            prev = (j, dn)
        for t in norm_thunks(prev[0], prev[1]):
            t()

    nc.compile()
    return nc


def _prep_in_maps(in_features, qkv_proj_weight, o_proj_weight):
    """Per-core input dict (host-side shard + transpose + cast)."""
    # causal 0/1 triangle for diagonal 128x128 blocks
    r = np.arange(128)[:, None]
    c = np.arange(128)[None, :]
    mk = (r <= c).astype(np.float32).astype(BF16)
    sel = np.zeros((GH, GH * 64), np.float32)
    for h in range(GH):
        sel[h, h * 64:(h + 1) * 64] = 1.0

    scale = 1.0 / np.sqrt(np.float32(DK))
    in_maps = []
    for core in range(NCORES):
        b, g = core // 2, core % 2
        sl = slice(g * GO, (g + 1) * GO)
        xT = np.ascontiguousarray(in_features[b].T).astype(BF16)
        wqT = np.ascontiguousarray((qkv_proj_weight[0][sl, :] * scale).T
                                   ).astype(BF16)
        wkT = np.ascontiguousarray(qkv_proj_weight[1][sl, :].T).astype(BF16)
        wvT = np.ascontiguousarray(qkv_proj_weight[2][sl, :].T).astype(BF16)
        owT = np.ascontiguousarray(o_proj_weight[:, sl].T).astype(BF16)
        in_maps.append({"xT": xT, "wqT": wqT, "wkT": wkT, "wvT": wvT,
                        "owT": owT, "mk": mk, "sel": sel})
    return in_maps


def _get_runner():
    """Persistent sharded-jit runner over the 8 NeuronCores.

    Mirrors bass_utils.run_bass_kernel_spmd's axon path
    (bass2jax.run_bass_via_pjrt), but keeps the jitted executable cached
    across calls and skips install_neuronx_cc_hook: under axon the
    bass_exec custom-call is compiled terminal-side, and the client-side
    hook rejects the SPMD-partitioned HLO.
    """
    if "runner" in _CACHE:
        return _CACHE["runner"]

    import jax
    from concourse import mybir
    from concourse.bass2jax import _bass_exec_p, partition_id_tensor
    from jax.sharding import Mesh, PartitionSpec
    from jax.experimental.shard_map import shard_map

    nc = _build_bass()

    partition_name = (nc.partition_id_tensor.name
                      if nc.partition_id_tensor else None)
    in_names, out_names, out_avals, zero_outs = [], [], [], []
    for alloc in nc.m.functions[0].allocations:
        if not isinstance(alloc, mybir.MemoryLocationSet):
            continue
        name = alloc.memorylocations[0].name
        if alloc.kind == "ExternalInput":
            if name != partition_name:
                in_names.append(name)
        elif alloc.kind == "ExternalOutput":
            out_names.append(name)
            shape = tuple(alloc.tensor_shape)
            dtype = mybir.dt.np(alloc.dtype)
            out_avals.append(jax.core.ShapedArray(shape, dtype))
            zero_outs.append(np.zeros(shape, dtype))
    n_params = len(in_names)
    n_outs = len(out_avals)
    all_in = list(in_names) + out_names + (
        [partition_name] if partition_name else [])

    def _body(*args):
        operands = list(args)
        if partition_name is not None:
            operands.append(partition_id_tensor())
        return tuple(_bass_exec_p.bind(
            *operands,
            out_avals=tuple(out_avals),
            in_names=tuple(all_in),
            out_names=tuple(out_names),
            lowering_input_output_aliases=(),
            sim_require_finite=True, sim_require_nnan=True, nc=nc))

    devices = jax.devices()[:NCORES]
    mesh = Mesh(np.asarray(devices), ("core",))
    fn = jax.jit(
        shard_map(_body, mesh=mesh,
                  in_specs=(PartitionSpec("core"),) * (n_params + n_outs),
                  out_specs=(PartitionSpec("core"),) * n_outs,
                  check_rep=False),
        donate_argnums=tuple(range(n_params, n_params + n_outs)),
        keep_unused=True)

    def run(in_maps):
        per_core = [[np.asarray(m[n]) for n in in_names] for m in in_maps]
        concat_in = [np.concatenate([per_core[c][i] for c in range(NCORES)],
                                    axis=0) for i in range(n_params)]
        concat_zeros = [np.zeros((NCORES * z.shape[0], *z.shape[1:]), z.dtype)
                        for z in zero_outs]
        out_arrs = fn(*concat_in, *concat_zeros)
        return np.asarray(out_arrs[out_names.index("part")]).reshape(
            NCORES, S, D)

    _CACHE["nc"] = nc
    _CACHE["runner"] = run
    return run


def kernel(in_features, qkv_proj_weight, o_proj_weight):
    run = _get_runner()
    in_maps = _prep_in_maps(np.asarray(in_features, np.float32),
                            np.asarray(qkv_proj_weight, np.float32),
                            np.asarray(o_proj_weight, np.float32))
    parts = run(in_maps)
    out = np.empty((B, S, D), np.float32)
    for b in range(B):
        out[b] = parts[2 * b] + parts[2 * b + 1]
    return out


# revision 23
# speedup vs baseline: 1.0116x; 1.0029x over previous
"""Causal multi-head self-attention on 8 trn2 NeuronCores.

Sharding: core c = (batch b = c//2, head-group g = c%2). Each core handles one
batch element and 6 of the 12 heads: QKV projection for its 384 output dims,
causal attention for its 6 heads, and a partial output projection against the
matching 384 columns of o_proj. Host sums the two partials per batch.

Device-side layout (per core):
  xT  [768, 2048]   x transposed (host-side), d on partitions
  QT/KT pair tiles [128, 2048]: partitions = (head 2p | head 2p+1) x dk=64,
    free dim = sequence. Produced by out = wT.T @ xT matmuls.
  S^T tiles [k, q]: scores transposed, computed with head-pair row tiling
    (K=dk=64 per head, two heads in array rows 0-63 / 64-127).
  exp on ScalarE (PSUM -> SBUF, bf16), causal mask applied as a 0/1 multiply
    on the diagonal blocks only.
  V_aug [k, 65]: V for one head + ones column; A@V matmul then yields both
    O^T (rows 0..63) and the softmax denominator (row 64) in one chain.
  Normalization: denom reciprocal (DVE), broadcast across partitions via a
    K=1 ones matmul on PE, multiply on DVE -> OT tiles [c, s].
  Output projection: out = OT.T @ owT accumulated over the 3 c-blocks.
"""

import numpy as np
import ml_dtypes

B, S, D = 4, 2048, 768
H, DK = 12, 64
NCORES = 8
GH = 6        # heads per core
GO = GH * DK  # 384, per-core slice of the qkv output dim
NP = 3        # head pairs per core
NSB = S // 128   # 16 sequence blocks of 128
NJ = S // 512    # 4 q-chunks of 512

BF16 = ml_dtypes.bfloat16

_CACHE = {}


def _build_bass():
    import concourse.bass as bass  # noqa: F401
    import concourse.tile as tile
    from concourse import bacc, mybir
    from contextlib import ExitStack

    f32 = mybir.dt.float32
    bf16 = mybir.dt.bfloat16
    AF = mybir.ActivationFunctionType

    nc = bacc.Bacc("TRN2", target_bir_lowering=False, debug=False,
                   num_devices=NCORES)

    xT_d = nc.dram_tensor("xT", [D, S], bf16, kind="ExternalInput").ap()
    wqT_d = nc.dram_tensor("wqT", [D, GO], bf16, kind="ExternalInput").ap()
    wkT_d = nc.dram_tensor("wkT", [D, GO], bf16, kind="ExternalInput").ap()
    wvT_d = nc.dram_tensor("wvT", [D, GO], bf16, kind="ExternalInput").ap()
    owT_d = nc.dram_tensor("owT", [GO, D], bf16, kind="ExternalInput").ap()
    mk_d = nc.dram_tensor("mk", [128, 128], bf16, kind="ExternalInput").ap()
    sel_d = nc.dram_tensor("sel", [GH, GH * 64], f32, kind="ExternalInput").ap()
    part_d = nc.dram_tensor("part", [S, D], f32, kind="ExternalOutput").ap()

    ND = D // 128  # 6 d-blocks

    with tile.TileContext(nc) as tc, ExitStack() as ctx:
        pers = ctx.enter_context(tc.tile_pool(name="pers", bufs=1))

        # ---- persistent SBUF tiles -------------------------------------
        xT = [pers.tile([128, S], bf16, tag=f"xT{d}", name=f"xT{d}") for d in range(ND)]
        wq = [pers.tile([128, GO], bf16, tag=f"wq{d}", name=f"wq{d}") for d in range(ND)]
        wk = [pers.tile([128, GO], bf16, tag=f"wk{d}", name=f"wk{d}") for d in range(ND)]
        wv = [pers.tile([128, GO], bf16, tag=f"wv{d}", name=f"wv{d}") for d in range(ND)]
        ow = [pers.tile([128, D], bf16, tag=f"ow{c}", name=f"ow{c}") for c in range(NP)]
        mkt = pers.tile([128, 128], bf16, tag="mkt", name="mkt")
        QT = [pers.tile([128, S], bf16, tag=f"QT{p}", name=f"QT{p}") for p in range(NP)]
        KT = [pers.tile([128, S], bf16, tag=f"KT{p}", name=f"KT{p}") for p in range(NP)]
        OT = [pers.tile([128, S], bf16, tag=f"OT{p}", name=f"OT{p}") for p in range(NP)]
        vaug = [pers.tile([128, GH * 65], bf16, tag=f"va{kb}", name=f"va{kb}")
                for kb in range(NSB)]
        # selector matrix (host-built): sel_t[k, h*64+m] = (k == h); the
        # [6, 64] slice for head h is the lhsT of a K=6 matmul that
        # broadcasts reciprocal row h across 64 output partitions
        sel_t = pers.tile([GH, GH * 64], f32, tag="sel_t", name="sel_t")
        nc.sync.dma_start(sel_t[:], sel_d[:])

        # DMA priority: what the first score matmuls need lands first —
        # wq/wk, then the j=3 x columns, then the rest in reverse-j order.
        for d in range(ND):
            nc.sync.dma_start(wq[d][:], wqT_d[d * 128:(d + 1) * 128, :])
            nc.sync.dma_start(wk[d][:], wkT_d[d * 128:(d + 1) * 128, :])
            nc.gpsimd.dma_start(xT[d][:, 3 * 512:4 * 512],
                                xT_d[d * 128:(d + 1) * 128, 3 * 512:4 * 512])
        for d in range(ND):
            nc.gpsimd.dma_start(wv[d][:], wvT_d[d * 128:(d + 1) * 128, :])
        nc.sync.dma_start(mkt[:], mk_d[:])
        for jj in (2, 1, 0):
            for d in range(ND):
                nc.sync.dma_start(xT[d][:, jj * 512:(jj + 1) * 512],
                                  xT_d[d * 128:(d + 1) * 128,
                                       jj * 512:(jj + 1) * 512])
        for c in range(NP):
            nc.gpsimd.dma_start(ow[c][:], owT_d[c * 128:(c + 1) * 128, :])

        # one unified PSUM pool: sp = scores strip (4 banks), u512 = shared
        # 1-bank slots for proj / AV / broadcast / output projection
        psp = ctx.enter_context(tc.tile_pool(name="psp", space="PSUM", bufs=2))
        pav = ctx.enter_context(tc.tile_pool(name="pav", space="PSUM", bufs=2))
        pu = ctx.enter_context(tc.tile_pool(name="pu", space="PSUM", bufs=2))
        expp = ctx.enter_context(tc.tile_pool(name="expp", bufs=4))
        sml = ctx.enter_context(tc.tile_pool(name="sml", bufs=2))
        stg = ctx.enter_context(tc.tile_pool(name="stg", bufs=3))

        def proj_qk(wsrc, dst, ob, j):
            ps = pu.tile([128, 512], f32, tag="u512", name="pjq")
            for d in range(ND):
                nc.tensor.matmul(ps[:], wsrc[d][:, ob * 128:(ob + 1) * 128],
                                 xT[d][:, j * 512:(j + 1) * 512],
                                 start=(d == 0), stop=(d == ND - 1))
            nc.vector.tensor_copy(dst[ob][:, j * 512:(j + 1) * 512], ps[:])

        def proj_v(sb):
            ps = pu.tile([128, GO], f32, tag="u512", name="pjv")
            for d in range(ND):
                nc.tensor.matmul(ps[:], xT[d][:, sb * 128:(sb + 1) * 128],
                                 wv[d][:], start=(d == 0), stop=(d == ND - 1))
            vt = vaug[sb]
            dst = vt[:].rearrange("p (h c) -> p h c", h=GH)[:, :, 0:64]
            nc.vector.tensor_copy(
                dst, ps[:].rearrange("p (h c) -> p h c", h=GH))
            nc.vector.memset(
                vt[:].rearrange("p (h c) -> p h c", h=GH)[:, :, 64:65], 1.0)

        # ---- attention, software-pipelined --------------------------------
        # One serial exp chain on ScalarE is the backbone. Per kb-group:
        # two score matmuls (head pair, row-tiled) -> exp -> mask; the AV
        # matmuls for group kb-1 run while exp(kb) drains, so PE's in-order
        # stream never waits on the chain. Filler thunks (V/QK projections,
        # normalization, output projection) are paced between groups.
        def attention_pair(j, p, dn, filler):
            nkb = 4 * j + 4
            nfill = len(filler)
            stride = max(1, -(-nkb // (nfill + 1))) if nfill else nkb + 1
            av = [pav.tile([65, 512], f32, tag="av", name="av")
                  for _ in range(2)]
            pend = None
            fi = 0
            for kb in range(nkb):
                sp = psp.tile([128, 1024], f32, tag="sp", name="sp")
                ex = expp.tile([128, 1024], bf16, tag="ex", name="ex")
                for hh in range(2):
                    nc.tensor.matmul(
                        sp[:, hh * 512:(hh + 1) * 512],
                        KT[p][hh * 64:(hh + 1) * 64,
                              kb * 128:(kb + 1) * 128],
                        QT[p][hh * 64:(hh + 1) * 64,
                              j * 512:(j + 1) * 512],
                        start=True, stop=True,
                        tile_position=(hh * 64, 0))
                nc.scalar.activation(ex[:], sp[:], AF.Exp)
                v = kb - 4 * j
                if v >= 0:  # diagonal block: causal mask
                    for hh in range(2):
                        if v > 0:
                            nc.vector.memset(
                                ex[:, hh * 512:hh * 512 + 128 * v], 0.0)
                        tri = ex[:, hh * 512 + 128 * v:
                                 hh * 512 + 128 * (v + 1)]
                        nc.vector.tensor_mul(tri, tri, mkt[:])
                if pend is not None:
                    pkb, pex = pend
                    for hh in range(2):
                        nc.tensor.matmul(
                            av[hh][:],
                            vaug[pkb][:, (2 * p + hh) * 65:
                                      (2 * p + hh + 1) * 65],
                            pex[:, hh * 512:(hh + 1) * 512],
                            start=(pkb == 0), stop=False)
                if kb % stride == stride - 1 and fi < nfill:
                    filler[fi]()
                    fi += 1
                pend = (kb, ex)
            pkb, pex = pend
            for hh in range(2):
                nc.tensor.matmul(
                    av[hh][:],
                    vaug[pkb][:, (2 * p + hh) * 65:(2 * p + hh + 1) * 65],
                    pex[:, hh * 512:(hh + 1) * 512],
                    start=(pkb == 0), stop=True)
            while fi < nfill:
                filler[fi]()
                fi += 1
            for hh in range(2):
                h = 2 * p + hh
                nc.vector.tensor_copy(
                    OT[p][hh * 64:(hh + 1) * 64, j * 512:(j + 1) * 512],
                    av[hh][0:64, :])
                ds_ = sml.tile([1, 512], f32, tag="dstg", name="dstg",
                               bufs=6)
                nc.vector.tensor_copy(ds_[:], av[hh][64:65, :])
                nc.sync.dma_start(dn[h:h + 1, :], ds_[:])

        def norm_thunks(j, dn):
            rcp6 = sml.tile([6, 512], f32, tag="rcp6", name="rcp6")
            nc.vector.reciprocal(rcp6[:], dn[:])
            out = []

            def mk_bc(p):
                def f():
                    bc = pu.tile([128, 512], f32, tag="u512", name="bc")
                    for hh in range(2):
                        h = 2 * p + hh
                        nc.tensor.matmul(bc[hh * 64:(hh + 1) * 64, :],
                                         sel_t[:, h * 64:(h + 1) * 64],
                                         rcp6[:], start=True, stop=True,
                                         tile_position=(0, hh * 64))
                    nc.vector.tensor_mul(OT[p][:, j * 512:(j + 1) * 512],
                                         OT[p][:, j * 512:(j + 1) * 512],
                                         bc[:])
                return f

            def mk_op(sb):
                def f():
                    op1 = pu.tile([128, 512], f32, tag="u512", name="op1")
                    op2 = pu.tile([128, 256], f32, tag="u512", name="op2")
                    for cb in range(NP):
                        lhs = OT[cb][:, sb * 128:(sb + 1) * 128]
                        nc.tensor.matmul(op1[:], lhs, ow[cb][:, 0:512],
                                         start=(cb == 0), stop=(cb == NP - 1))
                        nc.tensor.matmul(op2[:], lhs, ow[cb][:, 512:768],
                                         start=(cb == 0), stop=(cb == NP - 1))
                    st = stg.tile([128, D], f32, tag="st", name="st")
                    nc.vector.tensor_copy(st[:, 0:512], op1[:])
                    nc.vector.tensor_copy(st[:, 512:768], op2[:])
                    nc.sync.dma_start(part_d[sb * 128:(sb + 1) * 128, :],
                                      st[:])
                return f

            for p in range(NP):
                out.append(mk_bc(p))
            for sb in range(4 * j, 4 * j + 4):
                out.append(mk_op(sb))
            return out

        proj_qk(wq, QT, 0, 3)
        for jj in reversed(range(NJ)):
            proj_qk(wk, KT, 0, jj)

        vthunks = [(lambda sb=sb: proj_v(sb)) for sb in range(NSB)]

        def qthunk(ob, jj):
            return lambda: proj_qk(wq, QT, ob, jj)

        def kthunks(ob):
            return [(lambda jj=jj: proj_qk(wk, KT, ob, jj))
                    for jj in reversed(range(NJ))]

        # filler schedule per attention call era (jj, p), eras in call order;
        # each Q-projection chunk is emitted one era before its consumer.
        fills = [[] for _ in range(12)]
        fills[0] = vthunks + kthunks(1) + [qthunk(1, 3)]
        fills[1] = kthunks(2) + [qthunk(2, 3)]
        fills[2] = [qthunk(0, 2)]
        qlate = {3: qthunk(1, 2), 4: qthunk(2, 2), 5: qthunk(0, 1),
                 6: qthunk(1, 1), 7: qthunk(2, 1), 8: qthunk(0, 0),
                 9: qthunk(1, 0), 10: qthunk(2, 0)}

        prev = None
        era = 0
        for idx, j in enumerate(reversed(range(NJ))):
            dn = sml.tile([6, 512], f32, tag="dn", name="dn")
            if idx > 0:
                nt = norm_thunks(prev[0], prev[1])
                fills[era] = nt[:3] + fills[era]
                fills[era + 1] = nt[3:5] + fills[era + 1]
                fills[era + 2] = nt[5:] + fills[era + 2]
            for p in range(NP):
                f = fills[era]
                if era in qlate:
                    f = f + [qlate[era]]
                attention_pair(j, p, dn, f)
                era += 1
            prev = (j, dn)
        for t in norm_thunks(prev[0], prev[1]):
            t()

    nc.compile()
    return nc


def _prep_in_maps(in_features, qkv_proj_weight, o_proj_weight):
    """Per-core input dict (host-side shard + transpose + cast)."""
    # causal 0/1 triangle for diagonal 128x128 blocks
    r = np.arange(128)[:, None]
    c = np.arange(128)[None, :]
    mk = (r <= c).astype(np.float32).astype(BF16)
    sel = np.zeros((GH, GH * 64), np.float32)
    for h in range(GH):
        sel[h, h * 64:(h + 1) * 64] = 1.0

    scale = 1.0 / np.sqrt(np.float32(DK))
    in_maps = []
    for core in range(NCORES):
        b, g = core // 2, core % 2
        sl = slice(g * GO, (g + 1) * GO)
        xT = np.ascontiguousarray(in_features[b].T).astype(BF16)
        wqT = np.ascontiguousarray((qkv_proj_weight[0][sl, :] * scale).T
                                   ).astype(BF16)
        wkT = np.ascontiguousarray(qkv_proj_weight[1][sl, :].T).astype(BF16)
        wvT = np.ascontiguousarray(qkv_proj_weight[2][sl, :].T).astype(BF16)
        owT = np.ascontiguousarray(o_proj_weight[:, sl].T).astype(BF16)
        in_maps.append({"xT": xT, "wqT": wqT, "wkT": wkT, "wvT": wvT,
                        "owT": owT, "mk": mk, "sel": sel})
    return in_maps


def _get_runner():
    """Persistent sharded-jit runner over the 8 NeuronCores.

    Mirrors bass_utils.run_bass_kernel_spmd's axon path
    (bass2jax.run_bass_via_pjrt), but keeps the jitted executable cached
    across calls and skips install_neuronx_cc_hook: under axon the
    bass_exec custom-call is compiled terminal-side, and the client-side
    hook rejects the SPMD-partitioned HLO.
    """
    if "runner" in _CACHE:
        return _CACHE["runner"]

    import jax
    from concourse import mybir
    from concourse.bass2jax import _bass_exec_p, partition_id_tensor
    from jax.sharding import Mesh, PartitionSpec
    from jax.experimental.shard_map import shard_map

    nc = _build_bass()

    partition_name = (nc.partition_id_tensor.name
                      if nc.partition_id_tensor else None)
    in_names, out_names, out_avals, zero_outs = [], [], [], []
    for alloc in nc.m.functions[0].allocations:
        if not isinstance(alloc, mybir.MemoryLocationSet):
            continue
        name = alloc.memorylocations[0].name
        if alloc.kind == "ExternalInput":
            if name != partition_name:
                in_names.append(name)
        elif alloc.kind == "ExternalOutput":
            out_names.append(name)
            shape = tuple(alloc.tensor_shape)
            dtype = mybir.dt.np(alloc.dtype)
            out_avals.append(jax.core.ShapedArray(shape, dtype))
            zero_outs.append(np.zeros(shape, dtype))
    n_params = len(in_names)
    n_outs = len(out_avals)
    all_in = list(in_names) + out_names + (
        [partition_name] if partition_name else [])

    def _body(*args):
        operands = list(args)
        if partition_name is not None:
            operands.append(partition_id_tensor())
        return tuple(_bass_exec_p.bind(
            *operands,
            out_avals=tuple(out_avals),
            in_names=tuple(all_in),
            out_names=tuple(out_names),
            lowering_input_output_aliases=(),
            sim_require_finite=True, sim_require_nnan=True, nc=nc))

    devices = jax.devices()[:NCORES]
    mesh = Mesh(np.asarray(devices), ("core",))
    fn = jax.jit(
        shard_map(_body, mesh=mesh,
                  in_specs=(PartitionSpec("core"),) * (n_params + n_outs),
                  out_specs=(PartitionSpec("core"),) * n_outs,
                  check_rep=False),
        donate_argnums=tuple(range(n_params, n_params + n_outs)),
        keep_unused=True)

    def run(in_maps):
        per_core = [[np.asarray(m[n]) for n in in_names] for m in in_maps]
        concat_in = [np.concatenate([per_core[c][i] for c in range(NCORES)],
                                    axis=0) for i in range(n_params)]
        concat_zeros = [np.zeros((NCORES * z.shape[0], *z.shape[1:]), z.dtype)
                        for z in zero_outs]
        out_arrs = fn(*concat_in, *concat_zeros)
        return np.asarray(out_arrs[out_names.index("part")]).reshape(
            NCORES, S, D)

    _CACHE["nc"] = nc
    _CACHE["runner"] = run
    return run


def kernel(in_features, qkv_proj_weight, o_proj_weight):
    run = _get_runner()
    in_maps = _prep_in_maps(np.asarray(in_features, np.float32),
                            np.asarray(qkv_proj_weight, np.float32),
                            np.asarray(o_proj_weight, np.float32))
    parts = run(in_maps)
    out = np.empty((B, S, D), np.float32)
    for b in range(B):
        out[b] = parts[2 * b] + parts[2 * b + 1]
    return out
